# revision 3
# baseline (speedup 1.0000x reference)
"""Distributed causal attention with RoPE for trn2 (8 NeuronCores).

Problem: B=2, S=2048, DIM=2048, H=16 heads, D=128.
  out = softmax(causal(rope(xq) @ rope(xk)^T / sqrt(D))) @ xv @ wo^T

Sharding: tensor-parallel over heads (4 heads/core) x data-parallel over
batch (2 groups of 4 cores).  Attention is fully local per core; the only
collective is a small AllGather (bf16) of per-core attention outputs within
each batch group, after which each core computes a disjoint column slice of
the output projection.

Layout strategy (on-device matmuls contract over the partition axis):
  - host feeds x^T and w^T so no on-device transposes are needed
  - Q,K are produced transposed ([d, s]) which is what both the scores
    matmul (S^T = K @ Q^T) and the PV matmul (U^T = V^T @ E) consume
  - softmax runs on transposed scores: exp on ACT, causal masking by
    multiply, row-sums via a ones-vector matmul, normalization applied
    after PV with a 1/r rank-1 broadcast matmul + elementwise multiply
  - compute dtype bf16 (fp32 PSUM accumulation), output fp32
"""

import math
import sys

sys.path.insert(0, "/opt/trn_rl_repo")

import numpy as np
import ml_dtypes

import concourse.bass as bass
import concourse.mybir as mybir
import concourse.tile as tile
from concourse import bacc
from concourse.bass_utils import run_bass_kernel_spmd

BF16 = mybir.dt.bfloat16
F32 = mybir.dt.float32

B, S, DIM, H, D = 2, 2048, 2048, 16, 128
NCORES, TP = 8, 4
HPC = H // TP            # heads per core = 4
HD = HPC * D             # local hidden slice = 512
OSL = DIM // TP          # output column slice per core = 512
GROUPS = [[0, 1, 2, 3], [4, 5, 6, 7]]
INV_SQRT_D = 1.0 / math.sqrt(D)

SBW = 512                # s-block width for projections
NSB = S // SBW           # 4
NKT = DIM // 128         # 16 contraction tiles for projections
NQB = S // 512           # 4 q-blocks per head
NST = S // 128           # 16 s sub-tiles

LAST_RESULT = None
_CACHED_NC = None


def _build():
    nc = bacc.Bacc("TRN2", target_bir_lowering=False, debug=False,
                   num_devices=NCORES)

    xT = nc.declare_dram_parameter("xT", [DIM, S], BF16, isOutput=False)
    wqT = nc.declare_dram_parameter("wqT", [DIM, HD], BF16, isOutput=False)
    wkT = nc.declare_dram_parameter("wkT", [DIM, HD], BF16, isOutput=False)
    wvT = nc.declare_dram_parameter("wvT", [DIM, HD], BF16, isOutput=False)
    woT = nc.declare_dram_parameter("woT", [DIM, OSL], BF16, isOutput=False)
    cosb = nc.declare_dram_parameter("cosb", [128, S], BF16, isOutput=False)
    sinb = nc.declare_dram_parameter("sinb", [128, S], BF16, isOutput=False)
    swp = nc.declare_dram_parameter("swp", [128, 128], BF16, isOutput=False)
    msk = nc.declare_dram_parameter("msk", [4, 128, 512], BF16, isOutput=False)
    out = nc.declare_dram_parameter("out", [S, OSL], F32, isOutput=True)

    with tile.TileContext(nc) as tc:
        with (
            tc.tile_pool(name="res", bufs=1) as res,
            tc.tile_pool(name="dram", bufs=1, space="DRAM") as dram,
        ):
            # ---- resident tiles -------------------------------------------
            wq_t = res.tile([128, NKT, HD], BF16)
            wk_t = res.tile([128, NKT, HD], BF16)
            wv_t = res.tile([128, NKT, HD], BF16)
            cos_t = res.tile([128, S], BF16)
            sin_t = res.tile([128, S], BF16)
            swp_t = res.tile([128, 128], BF16)
            msk_t = res.tile([128, 4, 512], BF16)
            ones_b = res.tile([128, 1], BF16)
            ones_f = res.tile([1, 128], F32)
            qT_t = res.tile([128, HPC, S], BF16)   # rope'd Q^T per head
            kT_t = res.tile([128, HPC, S], BF16)   # rope'd K^T per head
            v_t = res.tile([128, NST, HD], BF16)   # V natural [s, dv]
            ao_t = res.tile([128, HPC, S], BF16)   # attn out^T per head

            ag_in = dram.tile([HD, S], BF16)
            ag_out = dram.tile([TP * HD, S], BF16)

            for i in range(NKT):
                nc.sync.dma_start(out=wq_t[:, i, :], in_=wqT[i * 128:(i + 1) * 128, :])
                nc.sync.dma_start(out=wk_t[:, i, :], in_=wkT[i * 128:(i + 1) * 128, :])
                nc.sync.dma_start(out=wv_t[:, i, :], in_=wvT[i * 128:(i + 1) * 128, :])
            nc.sync.dma_start(out=cos_t[:], in_=cosb[:])
            nc.sync.dma_start(out=sin_t[:], in_=sinb[:])
            nc.sync.dma_start(out=swp_t[:], in_=swp[:])
            for j in range(4):
                nc.sync.dma_start(out=msk_t[:, j, :], in_=msk[j])
            nc.vector.memset(ones_b[:], 1.0)
            nc.vector.memset(ones_f[:], 1.0)

            # ---- phase A: QKV projections + RoPE --------------------------
            with (
                tc.tile_pool(name="xa", bufs=2) as xa,
                tc.tile_pool(name="ta", bufs=2) as ta,
                tc.tile_pool(name="psA", bufs=1, space="PSUM") as psA,
            ):
                for sb in range(NSB):
                    ssl = slice(sb * SBW, (sb + 1) * SBW)
                    xt = xa.tile([128, NKT, SBW], BF16, tag="xt", bufs=2)
                    for i in range(NKT):
                        nc.sync.dma_start(
                            out=xt[:, i, :],
                            in_=xT[i * 128:(i + 1) * 128, ssl])
                    # Q and K projections, transposed out [dq, s], with RoPE
                    for (w_t, o_t) in ((wq_t, qT_t), (wk_t, kT_t)):
                        for m in range(HPC):
                            pp = psA.tile([128, SBW], F32, tag="proj", bufs=2)
                            for i in range(NKT):
                                nc.tensor.matmul(
                                    pp[:],
                                    lhsT=w_t[:, i, m * 128:(m + 1) * 128],
                                    rhs=xt[:, i, :],
                                    start=(i == 0), stop=(i == NKT - 1))
                            raw = ta.tile([128, SBW], BF16, tag="raw", bufs=2)
                            nc.scalar.copy(raw[:], pp[:])
                            ps = psA.tile([128, SBW], F32, tag="sw", bufs=2)
                            nc.tensor.matmul(ps[:], lhsT=swp_t[:], rhs=raw[:],
                                             start=True, stop=True)
                            m1 = ta.tile([128, SBW], F32, tag="m1", bufs=2)
                            nc.vector.tensor_tensor(
                                out=m1[:], in0=raw[:], in1=cos_t[:, ssl],
                                op=mybir.AluOpType.mult)
                            m2 = ta.tile([128, SBW], F32, tag="m2", bufs=2)
                            nc.vector.tensor_tensor(
                                out=m2[:], in0=ps[:], in1=sin_t[:, ssl],
                                op=mybir.AluOpType.mult)
                            nc.vector.tensor_tensor(
                                out=o_t[:, m, ssl], in0=m1[:], in1=m2[:],
                                op=mybir.AluOpType.add)
                    # V in natural layout [s, dv]
                    for m in range(4):
                        pv = psA.tile([128, HD], F32, tag="proj", bufs=2)
                        for i in range(NKT):
                            nc.tensor.matmul(
                                pv[:],
                                lhsT=xt[:, i, m * 128:(m + 1) * 128],
                                rhs=wv_t[:, i, :],
                                start=(i == 0), stop=(i == NKT - 1))
                        nc.scalar.copy(v_t[:, 4 * sb + m, :], pv[:])

            # ---- phase B: attention per head ------------------------------
            with (
                tc.tile_pool(name="eb", bufs=1) as eb,
                tc.tile_pool(name="tb", bufs=2) as tb,
                tc.tile_pool(name="psB", bufs=1, space="PSUM") as psB,
            ):
                for h in range(HPC):
                    for qb in range(NQB):
                        qsl = slice(qb * 512, (qb + 1) * 512)
                        nkt = 4 * qb + 4   # causal: k-tiles 0 .. 4qb+3
                        e_tiles = []
                        for kt in range(nkt):
                            pk = psB.tile([128, 512], F32, tag="sc", bufs=2)
                            nc.tensor.matmul(
                                pk[:],
                                lhsT=kT_t[:, h, kt * 128:(kt + 1) * 128],
                                rhs=qT_t[:, h, qsl],
                                start=True, stop=True)
                            et = eb.tile([128, 512], BF16, tag=f"e{kt}", bufs=2)
                            nc.scalar.activation(
                                et[:], pk[:],
                                mybir.ActivationFunctionType.Exp,
                                scale=INV_SQRT_D)
                            if kt >= 4 * qb:    # diagonal: causal mask
                                nc.vector.tensor_tensor(
                                    out=et[:], in0=et[:],
                                    in1=msk_t[:, kt - 4 * qb, :],
                                    op=mybir.AluOpType.mult)
                            e_tiles.append(et)
                        pr = psB.tile([1, 512], F32, tag="r", bufs=2)
                        for kt in range(nkt):
                            nc.tensor.matmul(
                                pr[:], lhsT=ones_b[:], rhs=e_tiles[kt][:],
                                start=(kt == 0), stop=(kt == nkt - 1))
                        pu = psB.tile([128, 512], F32, tag="u", bufs=2)
                        for kt in range(nkt):
                            nc.tensor.matmul(
                                pu[:],
                                lhsT=v_t[:, kt, h * 128:(h + 1) * 128],
                                rhs=e_tiles[kt][:],
                                start=(kt == 0), stop=(kt == nkt - 1))
                        rinv = tb.tile([1, 512], F32, tag="rinv", bufs=2)
                        nc.vector.reciprocal(rinv[:], pr[:])
                        pb = psB.tile([128, 512], F32, tag="bc", bufs=1)
                        nc.tensor.matmul(pb[:], lhsT=ones_f[:], rhs=rinv[:],
                                         start=True, stop=True)
                        rb = tb.tile([128, 512], F32, tag="rb", bufs=2)
                        nc.scalar.copy(rb[:], pb[:])
                        nc.vector.tensor_tensor(
                            out=ao_t[:, h, qsl], in0=pu[:], in1=rb[:],
                            op=mybir.AluOpType.mult)

            # ---- phase C: AllGather + output projection -------------------
            for h in range(HPC):
                nc.sync.dma_start(out=ag_in[h * 128:(h + 1) * 128, :],
                                  in_=ao_t[:, h, :])
            nc.gpsimd.collective_compute(
                "AllGather",
                mybir.AluOpType.bypass,
                ins=[ag_in.opt()],
                outs=[ag_out.opt()],
                replica_groups=GROUPS,
            )
            with (
                tc.tile_pool(name="wc", bufs=1) as wc,
                tc.tile_pool(name="xc", bufs=2) as xc,
                tc.tile_pool(name="tco", bufs=2) as tco,
                tc.tile_pool(name="psC", bufs=1, space="PSUM") as psC,
            ):
                wo_t = wc.tile([128, NKT, OSL], BF16)
                for i in range(NKT):
                    nc.sync.dma_start(out=wo_t[:, i, :],
                                      in_=woT[i * 128:(i + 1) * 128, :])
                for ss in range(NSB):
                    agt = xc.tile([128, NKT, SBW], BF16, tag="agt", bufs=2)
                    for i in range(NKT):
                        nc.sync.dma_start(
                            out=agt[:, i, :],
                            in_=ag_out[i * 128:(i + 1) * 128,
                                       ss * SBW:(ss + 1) * SBW])
                    for st in range(4):
                        po = psC.tile([128, OSL], F32, tag="fo", bufs=4)
                        for i in range(NKT):
                            nc.tensor.matmul(
                                po[:],
                                lhsT=agt[:, i, st * 128:(st + 1) * 128],
                                rhs=wo_t[:, i, :],
                                start=(i == 0), stop=(i == NKT - 1))
                        ot = tco.tile([128, OSL], F32, tag="ot", bufs=3)
                        nc.vector.tensor_copy(ot[:], po[:])
                        r0 = ss * SBW + st * 128
                        nc.sync.dma_start(out=out[r0:r0 + 128, :], in_=ot[:])
    nc.compile()
    return nc


def _host_prep(x, wq, wk, wv, wo):
    """Build per-core input maps (host-side transposes + bf16 casts)."""
    bf = ml_dtypes.bfloat16
    # rope tables in the transposed [d, s] layout
    inv = 1.0 / (10000.0 ** (np.arange(0, D, 2, dtype=np.float64) / D))  # [64]
    ang = np.outer(np.arange(S, dtype=np.float64), inv)                  # [S, 64]
    cos = np.cos(ang).T        # [64, S]
    sin = np.sin(ang).T        # [64, S]
    cosb = np.repeat(cos, 2, axis=0).astype(np.float32)                  # [128, S]
    sinb = np.repeat(sin, 2, axis=0).astype(np.float32)
    sinb[0::2, :] *= -1.0      # even d rows: -sin ; odd rows: +sin

    swp_m = np.zeros((128, 128), np.float32)
    idx = np.arange(0, 128, 2)
    swp_m[idx, idx + 1] = 1.0
    swp_m[idx + 1, idx] = 1.0

    ki = np.arange(128)[:, None]
    qj = np.arange(512)[None, :]
    msk_m = np.stack([(j * 128 + ki <= qj).astype(np.float32) for j in range(4)])

    xT_b = [np.ascontiguousarray(x[b].T).astype(bf) for b in range(B)]
    cosb, sinb = cosb.astype(bf), sinb.astype(bf)
    swp_m, msk_m = swp_m.astype(bf), msk_m.astype(bf)

    in_maps = []
    for c in range(NCORES):
        b = c // TP
        r = c % TP
        hrows = slice(r * HD, (r + 1) * HD)      # this core's head rows
        ocols = slice(r * OSL, (r + 1) * OSL)    # this core's output columns
        in_maps.append({
            "xT": xT_b[b],
            "wqT": np.ascontiguousarray(wq[hrows].T).astype(bf),
            "wkT": np.ascontiguousarray(wk[hrows].T).astype(bf),
            "wvT": np.ascontiguousarray(wv[hrows].T).astype(bf),
            "woT": np.ascontiguousarray(wo[ocols, :].T).astype(bf),
            "cosb": cosb,
            "sinb": sinb,
            "swp": swp_m,
            "msk": msk_m,
        })
    return in_maps


def kernel(x, wq, wk, wv, wo):
    global LAST_RESULT, _CACHED_NC
    if _CACHED_NC is None:
        _CACHED_NC = _build()
    nc = _CACHED_NC
    in_maps = _host_prep(x, wq, wk, wv, wo)
    res = run_bass_kernel_spmd(nc, in_maps, core_ids=list(range(NCORES)))
    LAST_RESULT = res
    out = np.empty((B, S, DIM), np.float32)
    for c in range(NCORES):
        b = c // TP
        r = c % TP
        out[b, :, r * OSL:(r + 1) * OSL] = res.results[c]["out"]
    return out


# revision 4
# speedup vs baseline: 1.1881x; 1.1881x over previous
"""Distributed causal attention with RoPE for trn2 (8 NeuronCores).

Problem: B=2, S=2048, DIM=2048, H=16 heads, D=128.
  out = softmax(causal(rope(xq) @ rope(xk)^T / sqrt(D))) @ xv @ wo^T

Sharding: tensor-parallel over heads (4 heads/core) x data-parallel over
batch (2 groups of 4 cores).  Attention is fully local per core; the only
collective is an AllGather (bf16) of per-core attention outputs within each
batch group, chunked along the sequence so it overlaps attention compute;
each core then computes a disjoint column slice of the output projection.

Layout strategy (on-device matmuls contract over the partition axis):
  - host feeds x^T and w^T so no on-device transposes are needed
  - Q,K are produced transposed ([d, s]) which is what both the scores
    matmul (S^T = K @ Q^T) and the PV matmul (U^T = V^T @ E) consume
  - softmax runs on transposed scores: exp on ACT, causal masking by
    multiply, row-sums via a ones-vector matmul, normalization applied
    after PV with a 1/r rank-1 broadcast matmul + elementwise multiply
  - compute dtype bf16 (fp32 PSUM accumulation), output fp32
"""

import math
import sys

sys.path.insert(0, "/opt/trn_rl_repo")

import numpy as np
import ml_dtypes

import concourse.bass as bass
import concourse.mybir as mybir
import concourse.tile as tile
from concourse import bacc
from concourse.bass_utils import run_bass_kernel_spmd

BF16 = mybir.dt.bfloat16
F32 = mybir.dt.float32

B, S, DIM, H, D = 2, 2048, 2048, 16, 128
NCORES, TP = 8, 4
HPC = H // TP            # heads per core = 4
HD = HPC * D             # local hidden slice = 512
OSL = DIM // TP          # output column slice per core = 512
GROUPS = [[0, 1, 2, 3], [4, 5, 6, 7]]
INV_SQRT_D = 1.0 / math.sqrt(D)

SBW = 512                # s-block width
NSB = S // SBW           # 4
NKT = DIM // 128         # 16 contraction tiles for projections
NQB = S // 512           # 4 q-blocks per head

LAST_RESULT = None
_CACHED_NC = None


def _attention_block(nc, tc, qb, h, qT_t, kT_t, v_t, ao_t, msk_t, ones_b,
                     ones_f, eb, tb, psB):
    """Causal attention for one (q-block, head): writes ao_t[:, h, qsl]."""
    qsl = slice(qb * 512, (qb + 1) * 512)
    nkt = 4 * qb + 4     # causal: k-tiles 0 .. 4qb+3
    e_tiles = []
    for kt in range(nkt):
        pk = psB.tile([128, 512], F32, tag="sc", bufs=2, name=f"pk{qb}{h}{kt}")
        nc.tensor.matmul(
            pk[:],
            lhsT=kT_t[:, h, kt * 128:(kt + 1) * 128],
            rhs=qT_t[:, h, qsl],
            start=True, stop=True)
        et = eb.tile([128, 512], BF16, tag=f"e{kt}", bufs=2,
                     name=f"et{qb}{h}{kt}")
        nc.scalar.activation(
            et[:], pk[:], mybir.ActivationFunctionType.Exp, scale=INV_SQRT_D)
        if kt >= 4 * qb:     # diagonal tile: causal mask
            nc.vector.tensor_tensor(
                out=et[:], in0=et[:], in1=msk_t[:, kt - 4 * qb, :],
                op=mybir.AluOpType.mult)
        e_tiles.append(et)
    pr = psB.tile([1, 512], F32, tag="r", bufs=1, name=f"pr{qb}{h}")
    for kt in range(nkt):
        nc.tensor.matmul(pr[:], lhsT=ones_b[:], rhs=e_tiles[kt][:],
                         start=(kt == 0), stop=(kt == nkt - 1))
    pu = psB.tile([128, 512], F32, tag="u", bufs=2, name=f"pu{qb}{h}")
    for kt in range(nkt):
        nc.tensor.matmul(
            pu[:],
            lhsT=v_t[:, kt, h * 128:(h + 1) * 128],
            rhs=e_tiles[kt][:],
            start=(kt == 0), stop=(kt == nkt - 1))
    rinv = tb.tile([1, 512], F32, tag="rinv", bufs=2, name=f"ri{qb}{h}")
    nc.vector.reciprocal(rinv[:], pr[:])
    pb = psB.tile([128, 512], F32, tag="bc", bufs=1, name=f"pb{qb}{h}")
    nc.tensor.matmul(pb[:], lhsT=ones_f[:], rhs=rinv[:], start=True, stop=True)
    rb = tb.tile([128, 512], F32, tag="rb", bufs=2, name=f"rb{qb}{h}")
    nc.scalar.copy(rb[:], pb[:])
    nc.vector.tensor_tensor(
        out=ao_t[:, h, qsl], in0=pu[:], in1=rb[:], op=mybir.AluOpType.mult)


def _out_proj_block(nc, qb, agt_list, wo_t, out, tco, psC):
    """Output projection for s-chunk qb from gathered heads."""
    agt = agt_list[qb]
    for st in range(4):
        po = psC.tile([128, OSL], F32, tag="fo", bufs=2, name=f"po{qb}{st}")
        for i in range(NKT):
            nc.tensor.matmul(
                po[:],
                lhsT=agt[:, i, st * 128:(st + 1) * 128],
                rhs=wo_t[:, i, :],
                start=(i == 0), stop=(i == NKT - 1))
        ot = tco.tile([128, OSL], F32, tag="ot", bufs=3, name=f"ot{qb}{st}")
        nc.vector.tensor_copy(ot[:], po[:])
        r0 = qb * SBW + st * 128
        nc.sync.dma_start(out=out[r0:r0 + 128, :], in_=ot[:])


def _build():
    nc = bacc.Bacc("TRN2", target_bir_lowering=False, debug=False,
                   num_devices=NCORES)

    xT = nc.declare_dram_parameter("xT", [DIM, S], BF16, isOutput=False)
    wqT = nc.declare_dram_parameter("wqT", [DIM, HD], BF16, isOutput=False)
    wkT = nc.declare_dram_parameter("wkT", [DIM, HD], BF16, isOutput=False)
    wvT = nc.declare_dram_parameter("wvT", [DIM, HD], BF16, isOutput=False)
    woT = nc.declare_dram_parameter("woT", [DIM, OSL], BF16, isOutput=False)
    cosb = nc.declare_dram_parameter("cosb", [128, S], BF16, isOutput=False)
    sinb = nc.declare_dram_parameter("sinb", [128, S], BF16, isOutput=False)
    swp = nc.declare_dram_parameter("swp", [128, 128], BF16, isOutput=False)
    msk = nc.declare_dram_parameter("msk", [4, 128, 512], BF16, isOutput=False)
    out = nc.declare_dram_parameter("out", [S, OSL], F32, isOutput=True)

    with tile.TileContext(nc) as tc:
        with (
            tc.tile_pool(name="res", bufs=1) as res,
            tc.tile_pool(name="dram", bufs=1, space="DRAM") as dram,
        ):
            # ---- resident tiles (live through the whole kernel) -----------
            msk_t = res.tile([128, 4, 512], BF16)
            ones_b = res.tile([128, 1], BF16)
            ones_f = res.tile([1, 128], F32)
            qT_t = res.tile([128, HPC, S], BF16)   # rope'd Q^T per head
            kT_t = res.tile([128, HPC, S], BF16)   # rope'd K^T per head
            v_t = res.tile([128, NSB * 4, HD], BF16)  # V natural [s, dv]
            ao_t = res.tile([128, HPC, S], BF16)   # attn out^T per head
            wo_t = res.tile([128, NKT, OSL], BF16)

            # ---- phase A: QKV projections + RoPE --------------------------
            # weight pool is closed after phase A to free SBUF for B/C
            with (
                tc.tile_pool(name="wA", bufs=1) as wA,
                tc.tile_pool(name="xa", bufs=2) as xa,
                tc.tile_pool(name="ta", bufs=2) as ta,
                tc.tile_pool(name="psA", bufs=1, space="PSUM") as psA,
            ):
                wq_t = wA.tile([128, NKT, HD], BF16)
                wk_t = wA.tile([128, NKT, HD], BF16)
                wv_t = wA.tile([128, NKT, HD], BF16)
                cos_t = wA.tile([128, S], BF16)
                sin_t = wA.tile([128, S], BF16)
                swp_t = wA.tile([128, 128], BF16)

                # DMA issue order matters: first projection needs wq + xt(0)
                for i in range(NKT):
                    nc.sync.dma_start(out=wq_t[:, i, :],
                                      in_=wqT[i * 128:(i + 1) * 128, :])
                xts = []
                for sb in range(NSB):
                    xts.append(xa.tile([128, NKT, SBW], BF16, tag="xt", bufs=2,
                                       name=f"xt{sb}"))
                for i in range(NKT):
                    nc.sync.dma_start(out=xts[0][:, i, :],
                                      in_=xT[i * 128:(i + 1) * 128, 0:SBW])
                for i in range(NKT):
                    nc.sync.dma_start(out=wk_t[:, i, :],
                                      in_=wkT[i * 128:(i + 1) * 128, :])
                nc.sync.dma_start(out=cos_t[:], in_=cosb[:])
                nc.sync.dma_start(out=sin_t[:], in_=sinb[:])
                nc.sync.dma_start(out=swp_t[:], in_=swp[:])
                for i in range(NKT):
                    nc.sync.dma_start(out=wv_t[:, i, :],
                                      in_=wvT[i * 128:(i + 1) * 128, :])
                for j in range(4):
                    nc.sync.dma_start(out=msk_t[:, j, :], in_=msk[j])
                for i in range(NKT):
                    nc.sync.dma_start(out=wo_t[:, i, :],
                                      in_=woT[i * 128:(i + 1) * 128, :])
                nc.vector.memset(ones_b[:], 1.0)
                nc.vector.memset(ones_f[:], 1.0)

                for sb in range(NSB):
                    ssl = slice(sb * SBW, (sb + 1) * SBW)
                    xt = xts[sb]
                    if sb > 0:
                        for i in range(NKT):
                            nc.sync.dma_start(
                                out=xt[:, i, :],
                                in_=xT[i * 128:(i + 1) * 128, ssl])
                    # Q and K projections -> transposed [dq, s], with RoPE
                    for (w_t, o_t) in ((wq_t, qT_t), (wk_t, kT_t)):
                        for m in range(HPC):
                            pp = psA.tile([128, SBW], F32, tag="proj", bufs=2,
                                          name=f"pp{sb}{m}")
                            for i in range(NKT):
                                nc.tensor.matmul(
                                    pp[:],
                                    lhsT=w_t[:, i, m * 128:(m + 1) * 128],
                                    rhs=xt[:, i, :],
                                    start=(i == 0), stop=(i == NKT - 1))
                            raw = ta.tile([128, SBW], BF16, tag="raw", bufs=2,
                                          name=f"raw{sb}{m}")
                            nc.scalar.copy(raw[:], pp[:])
                            ps = psA.tile([128, SBW], F32, tag="sw", bufs=2,
                                          name=f"psw{sb}{m}")
                            nc.tensor.matmul(ps[:], lhsT=swp_t[:], rhs=raw[:],
                                             start=True, stop=True)
                            m1 = ta.tile([128, SBW], F32, tag="m1", bufs=2,
                                         name=f"m1_{sb}{m}")
                            nc.vector.tensor_tensor(
                                out=m1[:], in0=raw[:], in1=cos_t[:, ssl],
                                op=mybir.AluOpType.mult)
                            m2 = ta.tile([128, SBW], F32, tag="m2", bufs=2,
                                         name=f"m2_{sb}{m}")
                            nc.vector.tensor_tensor(
                                out=m2[:], in0=ps[:], in1=sin_t[:, ssl],
                                op=mybir.AluOpType.mult)
                            nc.vector.tensor_tensor(
                                out=o_t[:, m, ssl], in0=m1[:], in1=m2[:],
                                op=mybir.AluOpType.add)
                    # V in natural layout [s, dv]
                    for m in range(4):
                        pv = psA.tile([128, HD], F32, tag="proj", bufs=2,
                                      name=f"pv{sb}{m}")
                        for i in range(NKT):
                            nc.tensor.matmul(
                                pv[:],
                                lhsT=xt[:, i, m * 128:(m + 1) * 128],
                                rhs=wv_t[:, i, :],
                                start=(i == 0), stop=(i == NKT - 1))
                        nc.scalar.copy(v_t[:, 4 * sb + m, :], pv[:])

            # ---- phases B+C: attention, chunked AllGather, out-projection -
            with (
                tc.tile_pool(name="eb", bufs=1) as eb,
                tc.tile_pool(name="tb", bufs=2) as tb,
                tc.tile_pool(name="xc", bufs=2) as xc,
                tc.tile_pool(name="tco", bufs=2) as tco,
                tc.tile_pool(name="psB", bufs=1, space="PSUM") as psB,
                tc.tile_pool(name="psC", bufs=1, space="PSUM") as psC,
            ):
                agt_list = []
                for qb in range(NQB):
                    for h in range(HPC):
                        _attention_block(nc, tc, qb, h, qT_t, kT_t, v_t, ao_t,
                                         msk_t, ones_b, ones_f, eb, tb, psB)
                    # AllGather this s-chunk of attention outputs (bf16)
                    qsl = slice(qb * 512, (qb + 1) * 512)
                    ag_in = dram.tile([HD, SBW], BF16, tag="agin", bufs=2,
                                      name=f"agin{qb}")
                    for h in range(HPC):
                        nc.sync.dma_start(out=ag_in[h * 128:(h + 1) * 128, :],
                                          in_=ao_t[:, h, qsl])
                    ag_out = dram.tile([TP * HD, SBW], BF16, tag="agout",
                                       bufs=2, name=f"agout{qb}")
                    nc.gpsimd.collective_compute(
                        "AllGather",
                        mybir.AluOpType.bypass,
                        ins=[ag_in.opt()],
                        outs=[ag_out.opt()],
                        replica_groups=GROUPS,
                    )
                    agt = xc.tile([128, NKT, SBW], BF16, tag="agt", bufs=2,
                                  name=f"agt{qb}")
                    for i in range(NKT):
                        nc.sync.dma_start(
                            out=agt[:, i, :],
                            in_=ag_out[i * 128:(i + 1) * 128, :])
                    agt_list.append(agt)
                    # output projection runs one chunk behind the gather
                    if qb >= 1:
                        _out_proj_block(nc, qb - 1, agt_list, wo_t, out, tco,
                                        psC)
                _out_proj_block(nc, NQB - 1, agt_list, wo_t, out, tco, psC)
    nc.compile()
    return nc


def _host_prep(x, wq, wk, wv, wo):
    """Build per-core input maps (host-side transposes + bf16 casts)."""
    bf = ml_dtypes.bfloat16
    # rope tables in the transposed [d, s] layout
    inv = 1.0 / (10000.0 ** (np.arange(0, D, 2, dtype=np.float64) / D))  # [64]
    ang = np.outer(np.arange(S, dtype=np.float64), inv)                  # [S, 64]
    cos = np.cos(ang).T        # [64, S]
    sin = np.sin(ang).T        # [64, S]
    cosb = np.repeat(cos, 2, axis=0).astype(np.float32)                  # [128, S]
    sinb = np.repeat(sin, 2, axis=0).astype(np.float32)
    sinb[0::2, :] *= -1.0      # even d rows: -sin ; odd rows: +sin

    swp_m = np.zeros((128, 128), np.float32)
    idx = np.arange(0, 128, 2)
    swp_m[idx, idx + 1] = 1.0
    swp_m[idx + 1, idx] = 1.0

    ki = np.arange(128)[:, None]
    qj = np.arange(512)[None, :]
    msk_m = np.stack([(j * 128 + ki <= qj).astype(np.float32) for j in range(4)])

    xT_b = [np.ascontiguousarray(x[b].T).astype(bf) for b in range(B)]
    cosb, sinb = cosb.astype(bf), sinb.astype(bf)
    swp_m, msk_m = swp_m.astype(bf), msk_m.astype(bf)

    in_maps = []
    for c in range(NCORES):
        b = c // TP
        r = c % TP
        hrows = slice(r * HD, (r + 1) * HD)      # this core's head rows
        ocols = slice(r * OSL, (r + 1) * OSL)    # this core's output columns
        in_maps.append({
            "xT": xT_b[b],
            "wqT": np.ascontiguousarray(wq[hrows].T).astype(bf),
            "wkT": np.ascontiguousarray(wk[hrows].T).astype(bf),
            "wvT": np.ascontiguousarray(wv[hrows].T).astype(bf),
            "woT": np.ascontiguousarray(wo[ocols, :].T).astype(bf),
            "cosb": cosb,
            "sinb": sinb,
            "swp": swp_m,
            "msk": msk_m,
        })
    return in_maps


def kernel(x, wq, wk, wv, wo):
    global LAST_RESULT, _CACHED_NC
    if _CACHED_NC is None:
        _CACHED_NC = _build()
    nc = _CACHED_NC
    in_maps = _host_prep(x, wq, wk, wv, wo)
    res = run_bass_kernel_spmd(nc, in_maps, core_ids=list(range(NCORES)))
    LAST_RESULT = res
    out = np.empty((B, S, DIM), np.float32)
    for c in range(NCORES):
        b = c // TP
        r = c % TP
        out[b, :, r * OSL:(r + 1) * OSL] = res.results[c]["out"]
    return out


# revision 5
# speedup vs baseline: 1.2682x; 1.0674x over previous
"""Distributed causal attention with RoPE for trn2 (8 NeuronCores).

Problem: B=2, S=2048, DIM=2048, H=16 heads, D=128.
  out = softmax(causal(rope(xq) @ rope(xk)^T / sqrt(D))) @ xv @ wo^T

Sharding: tensor-parallel over heads (4 heads/core) x data-parallel over
batch (2 groups of 4 cores).  Attention is fully local per core; the only
collective is an AllGather (bf16) of per-core attention outputs within each
batch group, chunked along the sequence so it overlaps attention compute;
each core then computes a disjoint column slice of the output projection.

Layout strategy (on-device matmuls contract over the partition axis):
  - host feeds x^T and w^T so no on-device transposes are needed
  - Q,K are produced transposed ([d, s]) which is what both the scores
    matmul (S^T = K @ Q^T) and the PV matmul (U^T = V^T @ E) consume
  - softmax runs on transposed scores: exp on ACT, causal masking by
    multiply, row-sums via a ones-vector matmul, normalization applied
    after PV with a 1/r rank-1 broadcast matmul + elementwise multiply
  - compute dtype bf16 (fp32 PSUM accumulation), output fp32
"""

import math
import sys

sys.path.insert(0, "/opt/trn_rl_repo")

import numpy as np
import ml_dtypes

import concourse.bass as bass
import concourse.mybir as mybir
import concourse.tile as tile
from concourse import bacc
from concourse.bass_utils import run_bass_kernel_spmd

BF16 = mybir.dt.bfloat16
F32 = mybir.dt.float32

B, S, DIM, H, D = 2, 2048, 2048, 16, 128
NCORES, TP = 8, 4
HPC = H // TP            # heads per core = 4
HD = HPC * D             # local hidden slice = 512
OSL = DIM // TP          # output column slice per core = 512
GROUPS = [[0, 1, 2, 3], [4, 5, 6, 7]]
INV_SQRT_D = 1.0 / math.sqrt(D)

SBW = 512                # s-block width
NSB = S // SBW           # 4
NKT = DIM // 128         # 16 contraction tiles for projections
NQB = S // 512           # 4 q-blocks per head

LAST_RESULT = None
_CACHED_NC = None


def _attention_block(nc, tc, qb, h, qT_t, kT_t, v_t, ao_t, msk_t, ones_m,
                     eb, tb, psB):
    """Causal attention for one (q-block, head): writes ao_t[:, h, qsl]."""
    qsl = slice(qb * 512, (qb + 1) * 512)
    nkt = 4 * qb + 4     # causal: k-tiles 0 .. 4qb+3
    e_tiles = []
    c0s = []
    for kt in range(nkt):
        j = kt - 4 * qb
        c0 = max(j, 0) * 128   # diagonal tiles: columns < c0 are fully masked
        nc0 = 512 - c0
        pk = psB.tile([128, 512], F32, tag="sc", bufs=2, name=f"pk{qb}{h}{kt}")
        nc.tensor.matmul(
            pk[:, c0:],
            lhsT=kT_t[:, h, kt * 128:(kt + 1) * 128],
            rhs=qT_t[:, h, qb * 512 + c0:(qb + 1) * 512],
            start=True, stop=True)
        et = eb.tile([128, 512], BF16, tag=f"e{kt}", bufs=2,
                     name=f"et{qb}{h}{kt}")
        nc.scalar.activation(
            et[:, c0:], pk[:, c0:], mybir.ActivationFunctionType.Exp,
            scale=INV_SQRT_D)
        if j >= 0:     # diagonal tile: causal mask
            nc.vector.tensor_tensor(
                out=et[:, c0:], in0=et[:, c0:], in1=msk_t[:, j, c0:],
                op=mybir.AluOpType.mult)
        e_tiles.append(et)
        c0s.append(c0)
    # row-sums r[q], replicated across all 128 partitions by a ones-matrix
    # matmul (the broadcast for the later normalization comes for free)
    pr = psB.tile([128, 512], F32, tag="r", bufs=2, name=f"pr{qb}{h}")
    for kt in range(nkt):
        c0 = c0s[kt]
        nc.tensor.matmul(pr[:, c0:], lhsT=ones_m[:], rhs=e_tiles[kt][:, c0:],
                         start=(kt == 0), stop=(kt == nkt - 1))
    pu = psB.tile([128, 512], F32, tag="u", bufs=2, name=f"pu{qb}{h}")
    for kt in range(nkt):
        c0 = c0s[kt]
        nc.tensor.matmul(
            pu[:, c0:],
            lhsT=v_t[:, kt, h * 128:(h + 1) * 128],
            rhs=e_tiles[kt][:, c0:],
            start=(kt == 0), stop=(kt == nkt - 1))
    rinv = tb.tile([128, 512], F32, tag="rinv", bufs=2, name=f"ri{qb}{h}")
    nc.vector.reciprocal(rinv[:], pr[:])
    nc.vector.tensor_tensor(
        out=ao_t[:, h, qsl], in0=pu[:], in1=rinv[:], op=mybir.AluOpType.mult)


def _out_proj_block(nc, qb, agt_list, wo_t, out, tco, psC):
    """Output projection for s-chunk qb from gathered heads."""
    agt = agt_list[qb]
    for st in range(4):
        po = psC.tile([128, OSL], F32, tag="fo", bufs=2, name=f"po{qb}{st}")
        for i in range(NKT):
            nc.tensor.matmul(
                po[:],
                lhsT=agt[:, i, st * 128:(st + 1) * 128],
                rhs=wo_t[:, i, :],
                start=(i == 0), stop=(i == NKT - 1))
        ot = tco.tile([128, OSL], F32, tag="ot", bufs=3, name=f"ot{qb}{st}")
        nc.vector.tensor_copy(ot[:], po[:])
        r0 = qb * SBW + st * 128
        nc.sync.dma_start(out=out[r0:r0 + 128, :], in_=ot[:])


def _build():
    nc = bacc.Bacc("TRN2", target_bir_lowering=False, debug=False,
                   num_devices=NCORES)

    xT = nc.declare_dram_parameter("xT", [DIM, S], BF16, isOutput=False)
    wqT = nc.declare_dram_parameter("wqT", [DIM, HD], BF16, isOutput=False)
    wkT = nc.declare_dram_parameter("wkT", [DIM, HD], BF16, isOutput=False)
    wvT = nc.declare_dram_parameter("wvT", [DIM, HD], BF16, isOutput=False)
    woT = nc.declare_dram_parameter("woT", [DIM, OSL], BF16, isOutput=False)
    cosb = nc.declare_dram_parameter("cosb", [128, S], BF16, isOutput=False)
    sinb = nc.declare_dram_parameter("sinb", [128, S], BF16, isOutput=False)
    swp = nc.declare_dram_parameter("swp", [128, 128], BF16, isOutput=False)
    msk = nc.declare_dram_parameter("msk", [4, 128, 512], BF16, isOutput=False)
    out = nc.declare_dram_parameter("out", [S, OSL], F32, isOutput=True)

    with tile.TileContext(nc) as tc:
        with (
            tc.tile_pool(name="res", bufs=1) as res,
            tc.tile_pool(name="dram", bufs=1, space="DRAM") as dram,
        ):
            # ---- resident tiles (live through the whole kernel) -----------
            msk_t = res.tile([128, 4, 512], BF16)
            ones_m = res.tile([128, 128], BF16)
            qT_t = res.tile([128, HPC, S], BF16)   # rope'd Q^T per head
            kT_t = res.tile([128, HPC, S], BF16)   # rope'd K^T per head
            v_t = res.tile([128, NSB * 4, HD], BF16)  # V natural [s, dv]
            ao_t = res.tile([128, HPC, S], BF16)   # attn out^T per head
            wo_t = res.tile([128, NKT, OSL], BF16)

            # ---- phase A: QKV projections + RoPE --------------------------
            # weight pool is closed after phase A to free SBUF for B/C
            with (
                tc.tile_pool(name="wA", bufs=1) as wA,
                tc.tile_pool(name="xa", bufs=2) as xa,
                tc.tile_pool(name="ta", bufs=2) as ta,
                tc.tile_pool(name="psA", bufs=1, space="PSUM") as psA,
            ):
                wq_t = wA.tile([128, NKT, HD], BF16)
                wk_t = wA.tile([128, NKT, HD], BF16)
                wv_t = wA.tile([128, NKT, HD], BF16)
                cos_t = wA.tile([128, S], BF16)
                sin_t = wA.tile([128, S], BF16)
                swp_t = wA.tile([128, 128], BF16)

                # DMA issue order matters: first projection needs wq + xt(0)
                for i in range(NKT):
                    nc.sync.dma_start(out=wq_t[:, i, :],
                                      in_=wqT[i * 128:(i + 1) * 128, :])
                xts = []
                for sb in range(NSB):
                    xts.append(xa.tile([128, NKT, SBW], BF16, tag="xt", bufs=2,
                                       name=f"xt{sb}"))
                for i in range(NKT):
                    nc.sync.dma_start(out=xts[0][:, i, :],
                                      in_=xT[i * 128:(i + 1) * 128, 0:SBW])
                for i in range(NKT):
                    nc.sync.dma_start(out=wk_t[:, i, :],
                                      in_=wkT[i * 128:(i + 1) * 128, :])
                nc.sync.dma_start(out=cos_t[:], in_=cosb[:])
                nc.sync.dma_start(out=sin_t[:], in_=sinb[:])
                nc.sync.dma_start(out=swp_t[:], in_=swp[:])
                for i in range(NKT):
                    nc.sync.dma_start(out=wv_t[:, i, :],
                                      in_=wvT[i * 128:(i + 1) * 128, :])
                for j in range(4):
                    nc.sync.dma_start(out=msk_t[:, j, :], in_=msk[j])
                for i in range(NKT):
                    nc.sync.dma_start(out=wo_t[:, i, :],
                                      in_=woT[i * 128:(i + 1) * 128, :])
                nc.vector.memset(ones_m[:], 1.0)

                for sb in range(NSB):
                    ssl = slice(sb * SBW, (sb + 1) * SBW)
                    xt = xts[sb]
                    if sb > 0:
                        for i in range(NKT):
                            nc.sync.dma_start(
                                out=xt[:, i, :],
                                in_=xT[i * 128:(i + 1) * 128, ssl])
                    # Q and K projections -> transposed [dq, s], with RoPE
                    for (w_t, o_t) in ((wq_t, qT_t), (wk_t, kT_t)):
                        for m in range(HPC):
                            pp = psA.tile([128, SBW], F32, tag="proj", bufs=2,
                                          name=f"pp{sb}{m}")
                            for i in range(NKT):
                                nc.tensor.matmul(
                                    pp[:],
                                    lhsT=w_t[:, i, m * 128:(m + 1) * 128],
                                    rhs=xt[:, i, :],
                                    start=(i == 0), stop=(i == NKT - 1))
                            raw = ta.tile([128, SBW], BF16, tag="raw", bufs=2,
                                          name=f"raw{sb}{m}")
                            nc.scalar.copy(raw[:], pp[:])
                            ps = psA.tile([128, SBW], F32, tag="sw", bufs=2,
                                          name=f"psw{sb}{m}")
                            nc.tensor.matmul(ps[:], lhsT=swp_t[:], rhs=raw[:],
                                             start=True, stop=True)
                            m1 = ta.tile([128, SBW], F32, tag="m1", bufs=2,
                                         name=f"m1_{sb}{m}")
                            nc.vector.tensor_tensor(
                                out=m1[:], in0=raw[:], in1=cos_t[:, ssl],
                                op=mybir.AluOpType.mult)
                            m2 = ta.tile([128, SBW], F32, tag="m2", bufs=2,
                                         name=f"m2_{sb}{m}")
                            nc.vector.tensor_tensor(
                                out=m2[:], in0=ps[:], in1=sin_t[:, ssl],
                                op=mybir.AluOpType.mult)
                            nc.vector.tensor_tensor(
                                out=o_t[:, m, ssl], in0=m1[:], in1=m2[:],
                                op=mybir.AluOpType.add)
                    # V in natural layout [s, dv]
                    for m in range(4):
                        pv = psA.tile([128, HD], F32, tag="proj", bufs=2,
                                      name=f"pv{sb}{m}")
                        for i in range(NKT):
                            nc.tensor.matmul(
                                pv[:],
                                lhsT=xt[:, i, m * 128:(m + 1) * 128],
                                rhs=wv_t[:, i, :],
                                start=(i == 0), stop=(i == NKT - 1))
                        nc.scalar.copy(v_t[:, 4 * sb + m, :], pv[:])

            # ---- phases B+C: attention, chunked AllGather, out-projection -
            with (
                tc.tile_pool(name="eb", bufs=1) as eb,
                tc.tile_pool(name="tb", bufs=2) as tb,
                tc.tile_pool(name="xc", bufs=2) as xc,
                tc.tile_pool(name="tco", bufs=2) as tco,
                tc.tile_pool(name="psB", bufs=1, space="PSUM") as psB,
                tc.tile_pool(name="psC", bufs=1, space="PSUM") as psC,
            ):
                agt_list = []
                for qb in range(NQB):
                    for h in range(HPC):
                        _attention_block(nc, tc, qb, h, qT_t, kT_t, v_t, ao_t,
                                         msk_t, ones_m, eb, tb, psB)
                    # AllGather this s-chunk of attention outputs (bf16)
                    qsl = slice(qb * 512, (qb + 1) * 512)
                    ag_in = dram.tile([HD, SBW], BF16, tag="agin", bufs=2,
                                      name=f"agin{qb}")
                    for h in range(HPC):
                        nc.sync.dma_start(out=ag_in[h * 128:(h + 1) * 128, :],
                                          in_=ao_t[:, h, qsl])
                    ag_out = dram.tile([TP * HD, SBW], BF16, tag="agout",
                                       bufs=2, name=f"agout{qb}")
                    nc.gpsimd.collective_compute(
                        "AllGather",
                        mybir.AluOpType.bypass,
                        ins=[ag_in.opt()],
                        outs=[ag_out.opt()],
                        replica_groups=GROUPS,
                    )
                    agt = xc.tile([128, NKT, SBW], BF16, tag="agt", bufs=2,
                                  name=f"agt{qb}")
                    for i in range(NKT):
                        nc.sync.dma_start(
                            out=agt[:, i, :],
                            in_=ag_out[i * 128:(i + 1) * 128, :])
                    agt_list.append(agt)
                    # output projection runs one chunk behind the gather
                    if qb >= 1:
                        _out_proj_block(nc, qb - 1, agt_list, wo_t, out, tco,
                                        psC)
                _out_proj_block(nc, NQB - 1, agt_list, wo_t, out, tco, psC)
    nc.compile()
    return nc


def _host_prep(x, wq, wk, wv, wo):
    """Build per-core input maps (host-side transposes + bf16 casts)."""
    bf = ml_dtypes.bfloat16
    # rope tables in the transposed [d, s] layout
    inv = 1.0 / (10000.0 ** (np.arange(0, D, 2, dtype=np.float64) / D))  # [64]
    ang = np.outer(np.arange(S, dtype=np.float64), inv)                  # [S, 64]
    cos = np.cos(ang).T        # [64, S]
    sin = np.sin(ang).T        # [64, S]
    cosb = np.repeat(cos, 2, axis=0).astype(np.float32)                  # [128, S]
    sinb = np.repeat(sin, 2, axis=0).astype(np.float32)
    sinb[0::2, :] *= -1.0      # even d rows: -sin ; odd rows: +sin

    swp_m = np.zeros((128, 128), np.float32)
    idx = np.arange(0, 128, 2)
    swp_m[idx, idx + 1] = 1.0
    swp_m[idx + 1, idx] = 1.0

    ki = np.arange(128)[:, None]
    qj = np.arange(512)[None, :]
    msk_m = np.stack([(j * 128 + ki <= qj).astype(np.float32) for j in range(4)])

    xT_b = [np.ascontiguousarray(x[b].T).astype(bf) for b in range(B)]
    cosb, sinb = cosb.astype(bf), sinb.astype(bf)
    swp_m, msk_m = swp_m.astype(bf), msk_m.astype(bf)

    in_maps = []
    for c in range(NCORES):
        b = c // TP
        r = c % TP
        hrows = slice(r * HD, (r + 1) * HD)      # this core's head rows
        ocols = slice(r * OSL, (r + 1) * OSL)    # this core's output columns
        in_maps.append({
            "xT": xT_b[b],
            "wqT": np.ascontiguousarray(wq[hrows].T).astype(bf),
            "wkT": np.ascontiguousarray(wk[hrows].T).astype(bf),
            "wvT": np.ascontiguousarray(wv[hrows].T).astype(bf),
            "woT": np.ascontiguousarray(wo[ocols, :].T).astype(bf),
            "cosb": cosb,
            "sinb": sinb,
            "swp": swp_m,
            "msk": msk_m,
        })
    return in_maps


def kernel(x, wq, wk, wv, wo):
    global LAST_RESULT, _CACHED_NC
    if _CACHED_NC is None:
        _CACHED_NC = _build()
    nc = _CACHED_NC
    in_maps = _host_prep(x, wq, wk, wv, wo)
    res = run_bass_kernel_spmd(nc, in_maps, core_ids=list(range(NCORES)))
    LAST_RESULT = res
    out = np.empty((B, S, DIM), np.float32)
    for c in range(NCORES):
        b = c // TP
        r = c % TP
        out[b, :, r * OSL:(r + 1) * OSL] = res.results[c]["out"]
    return out


# revision 6
# speedup vs baseline: 1.2760x; 1.0062x over previous
"""Distributed causal attention with RoPE for trn2 (8 NeuronCores).

Problem: B=2, S=2048, DIM=2048, H=16 heads, D=128.
  out = softmax(causal(rope(xq) @ rope(xk)^T / sqrt(D))) @ xv @ wo^T

Sharding: tensor-parallel over heads, 8-way: each core owns 2 global heads
for BOTH batches (4 local attention instances).  Attention is fully local;
the only collective is an 8-rank AllToAll per sequence chunk, which leaves
every core with the full per-batch attention output in global-head-major
row order (identical static indices on every core -> clean SPMD).  Each
core then computes one (batch, 512-column) slice of the output projection.

Layout strategy (on-device matmuls contract over the partition axis):
  - host feeds x^T and w^T so no on-device transposes are needed
  - Q,K are produced transposed ([d, s]) which is what both the scores
    matmul (S^T = K @ Q^T) and the PV matmul (U^T = V^T @ E) consume
  - softmax runs on transposed scores: exp on ACT, causal masking by
    multiply, row-sums via a ones-matrix matmul that also replicates the
    sum across partitions (the broadcast for normalization is free)
  - compute dtype bf16 (fp32 PSUM accumulation), output fp32
"""

import math
import sys

sys.path.insert(0, "/opt/trn_rl_repo")

import numpy as np
import ml_dtypes

import concourse.bass as bass
import concourse.mybir as mybir
import concourse.tile as tile
from concourse import bacc
from concourse.bass_utils import run_bass_kernel_spmd

BF16 = mybir.dt.bfloat16
F32 = mybir.dt.float32

B, S, DIM, H, D = 2, 2048, 2048, 16, 128
NCORES = 8
HPC = 2                  # global heads per core
NI = B * HPC             # local attention instances (batch x head) = 4
HD = HPC * D             # local hidden slice = 256
OSL = 512                # output column slice per core
GROUP = [list(range(NCORES))]
INV_SQRT_D = 1.0 / math.sqrt(D)

SBW = 512                # s-block width
NSB = S // SBW           # 4
NKT = DIM // 128         # 16 contraction tiles for projections
NQB = S // 512           # 4 q-blocks per instance

LAST_RESULT = None
_CACHED_NC = None


def _attention_block(nc, qb, hi, qT_t, kT_t, v_t, ao_t, msk_t, ones_m,
                     eb, tb, psB):
    """Causal attention for one (q-block, instance): writes ao_t[:, hi, qsl]."""
    qsl = slice(qb * 512, (qb + 1) * 512)
    b, j = hi // HPC, hi % HPC
    nkt = 4 * qb + 4     # causal: k-tiles 0 .. 4qb+3
    e_tiles, c0s = [], []
    for kt in range(nkt):
        dj = kt - 4 * qb
        c0 = max(dj, 0) * 128  # diagonal tiles: columns < c0 fully masked
        pk = psB.tile([128, 512], F32, tag="sc", bufs=2, name=f"pk{qb}{hi}{kt}")
        nc.tensor.matmul(
            pk[:, c0:],
            lhsT=kT_t[:, hi, kt * 128:(kt + 1) * 128],
            rhs=qT_t[:, hi, qb * 512 + c0:(qb + 1) * 512],
            start=True, stop=True)
        et = eb.tile([128, 512], BF16, tag=f"e{kt}", bufs=2,
                     name=f"et{qb}{hi}{kt}")
        nc.scalar.activation(
            et[:, c0:], pk[:, c0:], mybir.ActivationFunctionType.Exp,
            scale=INV_SQRT_D)
        if dj >= 0:     # diagonal tile: causal mask
            nc.vector.tensor_tensor(
                out=et[:, c0:], in0=et[:, c0:], in1=msk_t[:, dj, c0:],
                op=mybir.AluOpType.mult)
        e_tiles.append(et)
        c0s.append(c0)
    # row-sums r[q], replicated across all 128 partitions by a ones-matrix
    # matmul (the broadcast for the later normalization comes for free)
    pr = psB.tile([128, 512], F32, tag="r", bufs=2, name=f"pr{qb}{hi}")
    for kt in range(nkt):
        c0 = c0s[kt]
        nc.tensor.matmul(pr[:, c0:], lhsT=ones_m[:], rhs=e_tiles[kt][:, c0:],
                         start=(kt == 0), stop=(kt == nkt - 1))
    pu = psB.tile([128, 512], F32, tag="u", bufs=2, name=f"pu{qb}{hi}")
    for kt in range(nkt):
        c0 = c0s[kt]
        nc.tensor.matmul(
            pu[:, c0:],
            lhsT=v_t[:, b * 16 + kt, j * 128:(j + 1) * 128],
            rhs=e_tiles[kt][:, c0:],
            start=(kt == 0), stop=(kt == nkt - 1))
    rinv = tb.tile([128, 512], F32, tag="rinv", bufs=2, name=f"ri{qb}{hi}")
    nc.vector.reciprocal(rinv[:], pr[:])
    nc.vector.tensor_tensor(
        out=ao_t[:, hi, qsl], in0=pu[:], in1=rinv[:], op=mybir.AluOpType.mult)


def _out_proj_block(nc, qb, agt_list, wo_t, out, tco, psC):
    """Output projection for s-chunk qb from the AllToAll-delivered heads."""
    agt = agt_list[qb]
    for st in range(4):
        po = psC.tile([128, OSL], F32, tag="fo", bufs=2, name=f"po{qb}{st}")
        for i in range(NKT):
            nc.tensor.matmul(
                po[:],
                lhsT=agt[:, i, st * 128:(st + 1) * 128],
                rhs=wo_t[:, i, :],
                start=(i == 0), stop=(i == NKT - 1))
        ot = tco.tile([128, OSL], F32, tag="ot", bufs=3, name=f"ot{qb}{st}")
        nc.vector.tensor_copy(ot[:], po[:])
        r0 = qb * SBW + st * 128
        nc.sync.dma_start(out=out[r0:r0 + 128, :], in_=ot[:])


def _build():
    nc = bacc.Bacc("TRN2", target_bir_lowering=False, debug=False,
                   num_devices=NCORES)

    xT0 = nc.declare_dram_parameter("xT0", [DIM, S], BF16, isOutput=False)
    xT1 = nc.declare_dram_parameter("xT1", [DIM, S], BF16, isOutput=False)
    wqT = nc.declare_dram_parameter("wqT", [DIM, HD], BF16, isOutput=False)
    wkT = nc.declare_dram_parameter("wkT", [DIM, HD], BF16, isOutput=False)
    wvT = nc.declare_dram_parameter("wvT", [DIM, HD], BF16, isOutput=False)
    woT = nc.declare_dram_parameter("woT", [DIM, OSL], BF16, isOutput=False)
    cosb = nc.declare_dram_parameter("cosb", [128, S], BF16, isOutput=False)
    sinb = nc.declare_dram_parameter("sinb", [128, S], BF16, isOutput=False)
    swp = nc.declare_dram_parameter("swp", [128, 128], BF16, isOutput=False)
    msk = nc.declare_dram_parameter("msk", [4, 128, 512], BF16, isOutput=False)
    out = nc.declare_dram_parameter("out", [S, OSL], F32, isOutput=True)
    xTs = (xT0, xT1)

    with tile.TileContext(nc) as tc:
        with (
            tc.tile_pool(name="res", bufs=1) as res,
            tc.tile_pool(name="dram", bufs=1, space="DRAM") as dram,
        ):
            # ---- resident tiles (live through the whole kernel) -----------
            msk_t = res.tile([128, 4, 512], BF16)
            ones_m = res.tile([128, 128], BF16)
            qT_t = res.tile([128, NI, S], BF16)    # rope'd Q^T per instance
            kT_t = res.tile([128, NI, S], BF16)    # rope'd K^T per instance
            v_t = res.tile([128, B * 16, HD], BF16)  # V natural [s, dv] per b
            ao_t = res.tile([128, NI, S], BF16)    # attn out^T per instance
            wo_t = res.tile([128, NKT, OSL], BF16)

            # ---- phase A: QKV projections + RoPE --------------------------
            # weight pool is closed after phase A to free SBUF for B/C
            with (
                tc.tile_pool(name="wA", bufs=1) as wA,
                tc.tile_pool(name="xa", bufs=2) as xa,
                tc.tile_pool(name="ta", bufs=2) as ta,
                tc.tile_pool(name="psA", bufs=1, space="PSUM") as psA,
            ):
                wq_t = wA.tile([128, NKT, HD], BF16)
                wk_t = wA.tile([128, NKT, HD], BF16)
                wv_t = wA.tile([128, NKT, HD], BF16)
                cos_t = wA.tile([128, S], BF16)
                sin_t = wA.tile([128, S], BF16)
                swp_t = wA.tile([128, 128], BF16)

                # DMA issue order matters: first projection needs wq + xt(0,0)
                for i in range(NKT):
                    nc.sync.dma_start(out=wq_t[:, i, :],
                                      in_=wqT[i * 128:(i + 1) * 128, :])
                xt_tiles = {}
                for b in range(B):
                    for sb in range(NSB):
                        xt_tiles[(b, sb)] = xa.tile(
                            [128, NKT, SBW], BF16, tag="xt", bufs=2,
                            name=f"xt{b}{sb}")
                for i in range(NKT):
                    nc.sync.dma_start(out=xt_tiles[(0, 0)][:, i, :],
                                      in_=xT0[i * 128:(i + 1) * 128, 0:SBW])
                for i in range(NKT):
                    nc.sync.dma_start(out=wk_t[:, i, :],
                                      in_=wkT[i * 128:(i + 1) * 128, :])
                nc.sync.dma_start(out=cos_t[:], in_=cosb[:])
                nc.sync.dma_start(out=sin_t[:], in_=sinb[:])
                nc.sync.dma_start(out=swp_t[:], in_=swp[:])
                for i in range(NKT):
                    nc.sync.dma_start(out=wv_t[:, i, :],
                                      in_=wvT[i * 128:(i + 1) * 128, :])
                for j in range(4):
                    nc.sync.dma_start(out=msk_t[:, j, :], in_=msk[j])
                for i in range(NKT):
                    nc.sync.dma_start(out=wo_t[:, i, :],
                                      in_=woT[i * 128:(i + 1) * 128, :])
                nc.vector.memset(ones_m[:], 1.0)

                for b in range(B):
                    for sb in range(NSB):
                        ssl = slice(sb * SBW, (sb + 1) * SBW)
                        xt = xt_tiles[(b, sb)]
                        if (b, sb) != (0, 0):
                            for i in range(NKT):
                                nc.sync.dma_start(
                                    out=xt[:, i, :],
                                    in_=xTs[b][i * 128:(i + 1) * 128, ssl])
                        # Q and K projections -> transposed [dq, s] + RoPE
                        for (w_t, o_t) in ((wq_t, qT_t), (wk_t, kT_t)):
                            for j in range(HPC):
                                hi = B * 0 + b * HPC + j
                                pp = psA.tile([128, SBW], F32, tag="proj",
                                              bufs=2, name=f"pp{b}{sb}{j}")
                                for i in range(NKT):
                                    nc.tensor.matmul(
                                        pp[:],
                                        lhsT=w_t[:, i, j * 128:(j + 1) * 128],
                                        rhs=xt[:, i, :],
                                        start=(i == 0), stop=(i == NKT - 1))
                                raw = ta.tile([128, SBW], BF16, tag="raw",
                                              bufs=2, name=f"raw{b}{sb}{j}")
                                nc.scalar.copy(raw[:], pp[:])
                                ps = psA.tile([128, SBW], F32, tag="sw",
                                              bufs=2, name=f"psw{b}{sb}{j}")
                                nc.tensor.matmul(ps[:], lhsT=swp_t[:],
                                                 rhs=raw[:],
                                                 start=True, stop=True)
                                m1 = ta.tile([128, SBW], F32, tag="m1", bufs=2,
                                             name=f"m1_{b}{sb}{j}")
                                nc.vector.tensor_tensor(
                                    out=m1[:], in0=raw[:], in1=cos_t[:, ssl],
                                    op=mybir.AluOpType.mult)
                                m2 = ta.tile([128, SBW], F32, tag="m2", bufs=2,
                                             name=f"m2_{b}{sb}{j}")
                                nc.vector.tensor_tensor(
                                    out=m2[:], in0=ps[:], in1=sin_t[:, ssl],
                                    op=mybir.AluOpType.mult)
                                nc.vector.tensor_tensor(
                                    out=o_t[:, hi, ssl], in0=m1[:], in1=m2[:],
                                    op=mybir.AluOpType.add)
                        # V in natural layout [s, dv]
                        for m in range(4):
                            pv = psA.tile([128, HD], F32, tag="pv", bufs=2,
                                          name=f"pv{b}{sb}{m}")
                            for i in range(NKT):
                                nc.tensor.matmul(
                                    pv[:],
                                    lhsT=xt[:, i, m * 128:(m + 1) * 128],
                                    rhs=wv_t[:, i, :],
                                    start=(i == 0), stop=(i == NKT - 1))
                            nc.scalar.copy(v_t[:, b * 16 + 4 * sb + m, :],
                                           pv[:])

            # ---- phases B+C: attention, chunked AllToAll, out-projection --
            with (
                tc.tile_pool(name="eb", bufs=1) as eb,
                tc.tile_pool(name="tb", bufs=2) as tb,
                tc.tile_pool(name="xc", bufs=2) as xc,
                tc.tile_pool(name="tco", bufs=2) as tco,
                tc.tile_pool(name="psB", bufs=1, space="PSUM") as psB,
                tc.tile_pool(name="psC", bufs=1, space="PSUM") as psC,
            ):
                agt_list = []
                for qb in range(NQB):
                    for hi in range(NI):
                        _attention_block(nc, qb, hi, qT_t, kT_t, v_t, ao_t,
                                         msk_t, ones_m, eb, tb, psB)
                    # AllToAll this s-chunk: shard for destination rank d is
                    # this core's two heads of batch d//4 -> the output lands
                    # in global-head-major order on every core.
                    qsl = slice(qb * 512, (qb + 1) * 512)
                    ag_in = dram.tile([NCORES * HD, SBW], BF16, tag="agin",
                                      bufs=2, name=f"agin{qb}")
                    for d in range(NCORES):
                        bb = d // 4
                        for j in range(HPC):
                            r0 = d * HD + j * 128
                            nc.sync.dma_start(
                                out=ag_in[r0:r0 + 128, :],
                                in_=ao_t[:, bb * HPC + j, qsl])
                    ag_out = dram.tile([NCORES * HD, SBW], BF16, tag="agout",
                                       bufs=2, name=f"agout{qb}")
                    nc.gpsimd.collective_compute(
                        "AllToAll",
                        mybir.AluOpType.bypass,
                        ins=[ag_in.opt()],
                        outs=[ag_out.opt()],
                        replica_groups=GROUP,
                    )
                    agt = xc.tile([128, NKT, SBW], BF16, tag="agt", bufs=2,
                                  name=f"agt{qb}")
                    for i in range(NKT):
                        nc.sync.dma_start(
                            out=agt[:, i, :],
                            in_=ag_out[i * 128:(i + 1) * 128, :])
                    agt_list.append(agt)
                    # output projection runs one chunk behind the gather
                    if qb >= 1:
                        _out_proj_block(nc, qb - 1, agt_list, wo_t, out, tco,
                                        psC)
                _out_proj_block(nc, NQB - 1, agt_list, wo_t, out, tco, psC)
    nc.compile()
    return nc


def _host_prep(x, wq, wk, wv, wo):
    """Build per-core input maps (host-side transposes + bf16 casts)."""
    bf = ml_dtypes.bfloat16
    # rope tables in the transposed [d, s] layout
    inv = 1.0 / (10000.0 ** (np.arange(0, D, 2, dtype=np.float64) / D))  # [64]
    ang = np.outer(np.arange(S, dtype=np.float64), inv)                  # [S, 64]
    cos = np.cos(ang).T        # [64, S]
    sin = np.sin(ang).T        # [64, S]
    cosb = np.repeat(cos, 2, axis=0).astype(np.float32)                  # [128, S]
    sinb = np.repeat(sin, 2, axis=0).astype(np.float32)
    sinb[0::2, :] *= -1.0      # even d rows: -sin ; odd rows: +sin

    swp_m = np.zeros((128, 128), np.float32)
    idx = np.arange(0, 128, 2)
    swp_m[idx, idx + 1] = 1.0
    swp_m[idx + 1, idx] = 1.0

    ki = np.arange(128)[:, None]
    qj = np.arange(512)[None, :]
    msk_m = np.stack([(j * 128 + ki <= qj).astype(np.float32) for j in range(4)])

    xT_b = [np.ascontiguousarray(x[b].T).astype(bf) for b in range(B)]
    cosb, sinb = cosb.astype(bf), sinb.astype(bf)
    swp_m, msk_m = swp_m.astype(bf), msk_m.astype(bf)

    in_maps = []
    for c in range(NCORES):
        hrows = slice(c * HD, (c + 1) * HD)          # this core's 2 heads
        ocols = slice((c % 4) * OSL, (c % 4 + 1) * OSL)  # its output columns
        in_maps.append({
            "xT0": xT_b[0],
            "xT1": xT_b[1],
            "wqT": np.ascontiguousarray(wq[hrows].T).astype(bf),
            "wkT": np.ascontiguousarray(wk[hrows].T).astype(bf),
            "wvT": np.ascontiguousarray(wv[hrows].T).astype(bf),
            "woT": np.ascontiguousarray(wo[ocols, :].T).astype(bf),
            "cosb": cosb,
            "sinb": sinb,
            "swp": swp_m,
            "msk": msk_m,
        })
    return in_maps


def kernel(x, wq, wk, wv, wo):
    global LAST_RESULT, _CACHED_NC
    if _CACHED_NC is None:
        _CACHED_NC = _build()
    nc = _CACHED_NC
    in_maps = _host_prep(x, wq, wk, wv, wo)
    res = run_bass_kernel_spmd(nc, in_maps, core_ids=list(range(NCORES)))
    LAST_RESULT = res
    out = np.empty((B, S, DIM), np.float32)
    for c in range(NCORES):
        bb = c // 4
        csl = slice((c % 4) * OSL, (c % 4 + 1) * OSL)
        out[bb, :, csl] = res.results[c]["out"]
    return out


# revision 9
# speedup vs baseline: 1.4057x; 1.1017x over previous
"""Distributed causal attention with RoPE for trn2 (8 NeuronCores).

Problem: B=2, S=2048, DIM=2048, H=16 heads, D=128.
  out = softmax(causal(rope(xq) @ rope(xk)^T / sqrt(D))) @ xv @ wo^T

Sharding: tensor-parallel over heads, 8-way: each core owns 2 global heads
for BOTH batches (4 local attention instances).  Attention is fully local;
the only collective is an 8-rank AllToAll per sequence chunk, which leaves
every core with the full per-batch attention output in global-head-major
row order (identical static indices on every core -> clean SPMD).  Each
core then computes one (batch, 512-column) slice of the output projection.

Layout strategy (on-device matmuls contract over the partition axis):
  - host feeds x^T and w^T so no on-device transposes are needed
  - Q,K are produced transposed ([d, s]) which is what both the scores
    matmul (S^T = K @ Q^T) and the PV matmul (U^T = V^T @ E) consume
  - softmax runs on transposed scores: exp on ACT, causal masking by
    multiply, row-sums via a ones-matrix matmul that also replicates the
    sum across partitions (the broadcast for normalization is free)
  - compute dtype bf16 (fp32 PSUM accumulation), output fp32
"""

import math
import sys

sys.path.insert(0, "/opt/trn_rl_repo")

import numpy as np
import ml_dtypes

import concourse.bass as bass
import concourse.mybir as mybir
import concourse.tile as tile
from concourse import bacc
from concourse.bass_utils import run_bass_kernel_spmd

BF16 = mybir.dt.bfloat16
F32 = mybir.dt.float32

B, S, DIM, H, D = 2, 2048, 2048, 16, 128
NCORES = 8
HPC = 2                  # global heads per core
NI = B * HPC             # local attention instances (batch x head) = 4
HD = HPC * D             # local hidden slice = 256
OSL = 512                # output column slice per core
GROUP = [list(range(NCORES))]
INV_SQRT_D = 1.0 / math.sqrt(D)

SBW = 512                # s-block width
NSB = S // SBW           # 4
NKT = DIM // 128         # 16 contraction tiles for projections
NQB = S // 512           # 4 q-blocks per instance

LAST_RESULT = None
_CACHED_NC = None


def _attention_block(nc, qb, hi, qT_t, kT_t, v_t, ao_t, msk_t, ones_m,
                     eb, tb, psB):
    """Causal attention for one (q-block, instance): writes ao_t[:, hi, qsl]."""
    qsl = slice(qb * 512, (qb + 1) * 512)
    b, j = hi // HPC, hi % HPC
    nkt = 4 * qb + 4     # causal: k-tiles 0 .. 4qb+3
    e_tiles, c0s = [], []
    for kt in range(nkt):
        dj = kt - 4 * qb
        c0 = max(dj, 0) * 128  # diagonal tiles: columns < c0 fully masked
        pk = psB.tile([128, 512], F32, tag="sc", bufs=2, name=f"pk{qb}{hi}{kt}")
        nc.tensor.matmul(
            pk[:, c0:],
            lhsT=kT_t[:, hi, kt * 128:(kt + 1) * 128],
            rhs=qT_t[:, hi, qb * 512 + c0:(qb + 1) * 512],
            start=True, stop=True)
        et = eb.tile([128, 512], BF16, tag=f"e{kt}", bufs=2,
                     name=f"et{qb}{hi}{kt}")
        nc.scalar.activation(
            et[:, c0:], pk[:, c0:], mybir.ActivationFunctionType.Exp,
            scale=INV_SQRT_D)
        if dj >= 0:     # diagonal tile: causal mask
            nc.vector.tensor_tensor(
                out=et[:, c0:], in0=et[:, c0:], in1=msk_t[:, dj, c0:],
                op=mybir.AluOpType.mult)
        e_tiles.append(et)
        c0s.append(c0)
    # row-sums r[q], replicated across all 128 partitions by a ones-matrix
    # matmul (the broadcast for the later normalization comes for free)
    pr = psB.tile([128, 512], F32, tag="r", bufs=2, name=f"pr{qb}{hi}")
    for kt in range(nkt):
        c0 = c0s[kt]
        nc.tensor.matmul(pr[:, c0:], lhsT=ones_m[:], rhs=e_tiles[kt][:, c0:],
                         start=(kt == 0), stop=(kt == nkt - 1))
    pu = psB.tile([128, 512], F32, tag="u", bufs=2, name=f"pu{qb}{hi}")
    for kt in range(nkt):
        c0 = c0s[kt]
        nc.tensor.matmul(
            pu[:, c0:],
            lhsT=v_t[:, b * 16 + kt, j * 128:(j + 1) * 128],
            rhs=e_tiles[kt][:, c0:],
            start=(kt == 0), stop=(kt == nkt - 1))
    rinv = tb.tile([128, 512], F32, tag="rinv", bufs=2, name=f"ri{qb}{hi}")
    nc.vector.reciprocal(rinv[:], pr[:])
    nc.vector.tensor_tensor(
        out=ao_t[:, hi, qsl], in0=pu[:], in1=rinv[:], op=mybir.AluOpType.mult)


def _out_proj_block(nc, qb, agt_list, wo_t, out, tco, psC):
    """Output projection for s-chunk qb from the AllToAll-delivered heads."""
    agt = agt_list[qb]
    for st in range(4):
        po = psC.tile([128, OSL], F32, tag="fo", bufs=2, name=f"po{qb}{st}")
        for i in range(NKT):
            nc.tensor.matmul(
                po[:],
                lhsT=agt[:, i, st * 128:(st + 1) * 128],
                rhs=wo_t[:, i, :],
                start=(i == 0), stop=(i == NKT - 1))
        ot = tco.tile([128, OSL], F32, tag="ot", bufs=3, name=f"ot{qb}{st}")
        nc.vector.tensor_copy(ot[:], po[:])
        r0 = qb * 512 + st * 128
        nc.sync.dma_start(out=out[r0:r0 + 128, :], in_=ot[:])


def _build():
    nc = bacc.Bacc("TRN2", target_bir_lowering=False, debug=False,
                   num_devices=NCORES)

    xT0 = nc.declare_dram_parameter("xT0", [DIM, S], BF16, isOutput=False)
    xT1 = nc.declare_dram_parameter("xT1", [DIM, S], BF16, isOutput=False)
    wqT = nc.declare_dram_parameter("wqT", [DIM, HD], BF16, isOutput=False)
    wkT = nc.declare_dram_parameter("wkT", [DIM, HD], BF16, isOutput=False)
    wvT = nc.declare_dram_parameter("wvT", [DIM, HD], BF16, isOutput=False)
    woT = nc.declare_dram_parameter("woT", [DIM, OSL], BF16, isOutput=False)
    cosb = nc.declare_dram_parameter("cosb", [128, S], BF16, isOutput=False)
    sinb = nc.declare_dram_parameter("sinb", [128, S], BF16, isOutput=False)
    swp = nc.declare_dram_parameter("swp", [128, 128], BF16, isOutput=False)
    msk = nc.declare_dram_parameter("msk", [4, 128, 512], BF16, isOutput=False)
    out = nc.declare_dram_parameter("out", [S, OSL], F32, isOutput=True)
    xTs = (xT0, xT1)

    with tile.TileContext(nc) as tc:
        with (
            tc.tile_pool(name="res", bufs=1) as res,
            tc.tile_pool(name="dram", bufs=1, space="DRAM") as dram,
        ):
            # ---- resident tiles (live through the whole kernel) -----------
            msk_t = res.tile([128, 4, 512], BF16)
            ones_m = res.tile([128, 128], BF16)
            qT_t = res.tile([128, NI, S], BF16)    # rope'd Q^T per instance
            kT_t = res.tile([128, NI, S], BF16)    # rope'd K^T per instance
            v_t = res.tile([128, B * 16, HD], BF16)  # V natural [s, dv] per b
            ao_t = res.tile([128, NI, S], BF16)    # attn out^T per instance
            wo_t = res.tile([128, NKT, OSL], BF16)

            # ---- phase A: QKV projections + RoPE --------------------------
            # weight pool is closed after phase A to free SBUF for B/C
            with (
                tc.tile_pool(name="wA", bufs=1) as wA,
                tc.tile_pool(name="xa", bufs=2) as xa,
                tc.tile_pool(name="ta", bufs=2) as ta,
                tc.tile_pool(name="psA", bufs=1, space="PSUM") as psA,
            ):
                wq_t = wA.tile([128, NKT, HD], BF16)
                wk_t = wA.tile([128, NKT, HD], BF16)
                wv_t = wA.tile([128, NKT, HD], BF16)
                cos_t = wA.tile([128, S], BF16)
                sin_t = wA.tile([128, S], BF16)
                swp_t = wA.tile([128, 128], BF16)

                # DMA issue order matters: first projection needs wq + xt(0,0)
                for i in range(NKT):
                    nc.sync.dma_start(out=wq_t[:, i, :],
                                      in_=wqT[i * 128:(i + 1) * 128, :])
                xt_tiles = {}
                for b in range(B):
                    for sb in range(NSB):
                        xt_tiles[(b, sb)] = xa.tile(
                            [128, NKT, SBW], BF16, tag="xt", bufs=2,
                            name=f"xt{b}{sb}")
                for i in range(NKT):
                    nc.sync.dma_start(out=xt_tiles[(0, 0)][:, i, :],
                                      in_=xT0[i * 128:(i + 1) * 128, 0:SBW])
                for i in range(NKT):
                    nc.sync.dma_start(out=wk_t[:, i, :],
                                      in_=wkT[i * 128:(i + 1) * 128, :])
                nc.sync.dma_start(out=cos_t[:], in_=cosb[:])
                nc.sync.dma_start(out=sin_t[:], in_=sinb[:])
                nc.sync.dma_start(out=swp_t[:], in_=swp[:])
                for i in range(NKT):
                    nc.sync.dma_start(out=wv_t[:, i, :],
                                      in_=wvT[i * 128:(i + 1) * 128, :])
                for j in range(4):
                    nc.sync.dma_start(out=msk_t[:, j, :], in_=msk[j])
                for i in range(NKT):
                    nc.sync.dma_start(out=wo_t[:, i, :],
                                      in_=woT[i * 128:(i + 1) * 128, :])
                nc.vector.memset(ones_m[:], 1.0)

                for b in range(B):
                    for sb in range(NSB):
                        ssl = slice(sb * SBW, (sb + 1) * SBW)
                        xt = xt_tiles[(b, sb)]
                        if (b, sb) != (0, 0):
                            for i in range(NKT):
                                nc.sync.dma_start(
                                    out=xt[:, i, :],
                                    in_=xTs[b][i * 128:(i + 1) * 128, ssl])
                        # Q and K projections -> transposed [dq, s] + RoPE
                        for (w_t, o_t) in ((wq_t, qT_t), (wk_t, kT_t)):
                            for j in range(HPC):
                                hi = B * 0 + b * HPC + j
                                pp = psA.tile([128, SBW], F32, tag="proj",
                                              bufs=2, name=f"pp{b}{sb}{j}")
                                for i in range(NKT):
                                    nc.tensor.matmul(
                                        pp[:],
                                        lhsT=w_t[:, i, j * 128:(j + 1) * 128],
                                        rhs=xt[:, i, :],
                                        start=(i == 0), stop=(i == NKT - 1))
                                raw = ta.tile([128, SBW], BF16, tag="raw",
                                              bufs=2, name=f"raw{b}{sb}{j}")
                                nc.scalar.copy(raw[:], pp[:])
                                ps = psA.tile([128, SBW], F32, tag="sw",
                                              bufs=2, name=f"psw{b}{sb}{j}")
                                nc.tensor.matmul(ps[:], lhsT=swp_t[:],
                                                 rhs=raw[:],
                                                 start=True, stop=True)
                                m1 = ta.tile([128, SBW], F32, tag="m1", bufs=2,
                                             name=f"m1_{b}{sb}{j}")
                                nc.vector.tensor_tensor(
                                    out=m1[:], in0=raw[:], in1=cos_t[:, ssl],
                                    op=mybir.AluOpType.mult)
                                m2 = ta.tile([128, SBW], F32, tag="m2", bufs=2,
                                             name=f"m2_{b}{sb}{j}")
                                nc.vector.tensor_tensor(
                                    out=m2[:], in0=ps[:], in1=sin_t[:, ssl],
                                    op=mybir.AluOpType.mult)
                                nc.vector.tensor_tensor(
                                    out=o_t[:, hi, ssl], in0=m1[:], in1=m2[:],
                                    op=mybir.AluOpType.add)
                        # V in natural layout [s, dv]
                        for m in range(SBW // 128):
                            pv = psA.tile([128, HD], F32, tag="pv", bufs=2,
                                          name=f"pv{b}{sb}{m}")
                            for i in range(NKT):
                                nc.tensor.matmul(
                                    pv[:],
                                    lhsT=xt[:, i, m * 128:(m + 1) * 128],
                                    rhs=wv_t[:, i, :],
                                    start=(i == 0), stop=(i == NKT - 1))
                            nc.scalar.copy(v_t[:, b * 16 + (SBW // 128) * sb + m, :],
                                           pv[:])

            # ---- phases B+C: attention, chunked AllToAll, out-projection --
            with (
                tc.tile_pool(name="eb", bufs=1) as eb,
                tc.tile_pool(name="tb", bufs=2) as tb,
                tc.tile_pool(name="xc", bufs=2) as xc,
                tc.tile_pool(name="tco", bufs=2) as tco,
                tc.tile_pool(name="psB", bufs=1, space="PSUM") as psB,
                tc.tile_pool(name="psC", bufs=1, space="PSUM") as psC,
            ):
                agt_list = []
                for qb in range(NQB):
                    for hi in range(NI):
                        _attention_block(nc, qb, hi, qT_t, kT_t, v_t, ao_t,
                                         msk_t, ones_m, eb, tb, psB)
                    # AllToAll this s-chunk: shard for destination rank d is
                    # this core's two heads of batch d//4 -> the output lands
                    # in global-head-major order on every core.
                    qsl = slice(qb * 512, (qb + 1) * 512)
                    ag_in = dram.tile([NCORES * HD, 512], BF16, tag="agin",
                                      bufs=2, name=f"agin{qb}")
                    for d in range(NCORES):
                        bb = d // 4
                        for j in range(HPC):
                            r0 = d * HD + j * 128
                            nc.sync.dma_start(
                                out=ag_in[r0:r0 + 128, :],
                                in_=ao_t[:, bb * HPC + j, qsl])
                    ag_out = dram.tile([NCORES * HD, 512], BF16, tag="agout",
                                       bufs=2, name=f"agout{qb}")
                    nc.gpsimd.collective_compute(
                        "AllToAll",
                        mybir.AluOpType.bypass,
                        ins=[ag_in.opt()],
                        outs=[ag_out.opt()],
                        replica_groups=GROUP,
                    )
                    agt = xc.tile([128, NKT, 512], BF16, tag="agt", bufs=2,
                                  name=f"agt{qb}")
                    for i in range(NKT):
                        nc.sync.dma_start(
                            out=agt[:, i, :],
                            in_=ag_out[i * 128:(i + 1) * 128, :])
                    agt_list.append(agt)
                    # output projection runs two chunks behind the gather
                    if qb >= 2:
                        _out_proj_block(nc, qb - 2, agt_list, wo_t, out, tco,
                                        psC)
                _out_proj_block(nc, NQB - 2, agt_list, wo_t, out, tco, psC)
                _out_proj_block(nc, NQB - 1, agt_list, wo_t, out, tco, psC)
    nc.compile()
    return nc


def _host_prep(x, wq, wk, wv, wo):
    """Build per-core input maps (host-side transposes + bf16 casts)."""
    bf = ml_dtypes.bfloat16
    # rope tables in the transposed [d, s] layout
    inv = 1.0 / (10000.0 ** (np.arange(0, D, 2, dtype=np.float64) / D))  # [64]
    ang = np.outer(np.arange(S, dtype=np.float64), inv)                  # [S, 64]
    cos = np.cos(ang).T        # [64, S]
    sin = np.sin(ang).T        # [64, S]
    cosb = np.repeat(cos, 2, axis=0).astype(np.float32)                  # [128, S]
    sinb = np.repeat(sin, 2, axis=0).astype(np.float32)
    sinb[0::2, :] *= -1.0      # even d rows: -sin ; odd rows: +sin

    swp_m = np.zeros((128, 128), np.float32)
    idx = np.arange(0, 128, 2)
    swp_m[idx, idx + 1] = 1.0
    swp_m[idx + 1, idx] = 1.0

    ki = np.arange(128)[:, None]
    qj = np.arange(512)[None, :]
    msk_m = np.stack([(j * 128 + ki <= qj).astype(np.float32) for j in range(4)])

    xT_b = [np.ascontiguousarray(x[b].T).astype(bf) for b in range(B)]
    cosb, sinb = cosb.astype(bf), sinb.astype(bf)
    swp_m, msk_m = swp_m.astype(bf), msk_m.astype(bf)

    in_maps = []
    for c in range(NCORES):
        hrows = slice(c * HD, (c + 1) * HD)          # this core's 2 heads
        ocols = slice((c % 4) * OSL, (c % 4 + 1) * OSL)  # its output columns
        in_maps.append({
            "xT0": xT_b[0],
            "xT1": xT_b[1],
            "wqT": np.ascontiguousarray(wq[hrows].T).astype(bf),
            "wkT": np.ascontiguousarray(wk[hrows].T).astype(bf),
            "wvT": np.ascontiguousarray(wv[hrows].T).astype(bf),
            "woT": np.ascontiguousarray(wo[ocols, :].T).astype(bf),
            "cosb": cosb,
            "sinb": sinb,
            "swp": swp_m,
            "msk": msk_m,
        })
    return in_maps


def kernel(x, wq, wk, wv, wo):
    global LAST_RESULT, _CACHED_NC
    if _CACHED_NC is None:
        _CACHED_NC = _build()
    nc = _CACHED_NC
    in_maps = _host_prep(x, wq, wk, wv, wo)
    res = run_bass_kernel_spmd(nc, in_maps, core_ids=list(range(NCORES)))
    LAST_RESULT = res
    out = np.empty((B, S, DIM), np.float32)
    for c in range(NCORES):
        bb = c // 4
        csl = slice((c % 4) * OSL, (c % 4 + 1) * OSL)
        out[bb, :, csl] = res.results[c]["out"]
    return out
